# revision 21
# baseline (speedup 1.0000x reference)
"""DMN encoder (3-hop masked-attention message passing) on 8 trn2 cores.

Data-parallel over batch (16 rows/core). Per core, per batch row b:
  vs_n = V[b] @ wf  (hop-invariant attention projection)
  3 hops: num_h = mask*max(exp(vs)*exp(c_h), 1);  o_h = num_h^T V / denom_h
          u_{h+1} = relu(W u_h + bias) + o_h

Structure notes:
  - 3 o-passes (one per hop) instead of 2: removes the vu = V@wu
    per-neighbor projection entirely.
  - The reference's 1e-5*exp(max_att) denominator guard is dropped: the
    denominator always contains the max kept exponential, so the guard is
    bounded by ~1e-5 relative — far below the accuracy target.
  - vs is produced on whichever engine has slack when that row's V tile
    arrives: PE transpose+project (rows 0-3 early, 12-13 late), gpsimd
    multiply (rows 4,6,8,10) with the reduce on ACT/DVE, DVE for the rest.
  - o-pass matmuls are column-tiled 4-wide (tile_position=(0,32j)): four
    batch rows stream through the PE concurrently.
  - Hop scalars stay replicated across partitions (wu_rep / ones128
    matmuls) so the serial chain has no broadcast steps; tensor_scalar
    with AP scalars (TensorScalarPtr, ~9us on 128 partitions) is avoided.
"""
import sys

sys.path.insert(0, "/opt/trn_rl_repo")

import numpy as np
import concourse.bass as bass
import concourse.tile as tile
from concourse import mybir
from concourse.bass_utils import run_bass_kernel_spmd
from contextlib import ExitStack

N_CORES = 8
B, N, D = 128, 2048, 128
BC = B // N_CORES          # batch rows per core
CH = N // 128              # neighbor chunks of 128
AF = mybir.ActivationFunctionType
ALU = mybir.AluOpType
FP32 = mybir.dt.float32
BF16 = mybir.dt.bfloat16
CLAMP = 60.0               # overflow guard on exp() arguments

G = 4                       # rows per chain group
GROUPS = [(0, G), (4, G), (8, G), (12, G)]
PE_VS = (0, 1, 2, 3, 12, 13)   # rows: vs via PE transpose route
ACT_RED = (4, 6, 8, 10)        # rows: vs reduce via ACT accumulate
HALF_VS = (15,)                # rows: DVE vs in half-tile pieces (tail)

_mwctr = [0]


def _split_multiwaits(nc):
    """This walrus build rejects >1 sync-wait per instruction; hoist extras
    onto standalone EventSemaphore instructions on the same engine."""
    for fn in nc.m.functions:
        for bb in fn.blocks:
            new_list = []
            changed = False
            for ins in bb.instructions:
                si = getattr(ins, "sync_info", None)
                on_wait = list(si.on_wait) if si is not None else []
                if len(on_wait) > 1:
                    changed = True
                    for w in on_wait[:-1]:
                        _mwctr[0] += 1
                        ev = mybir.InstEventSemaphore(
                            name=f"I-mwfix-{_mwctr[0]}", ins=[], outs=[])
                        ev.engine = ins.engine
                        ev.debug = ins.debug
                        ev.sync_info = mybir.SyncInfo(on_wait=[w], on_update=[])
                        new_list.append(ev)
                        nc.register_instruction(ev, overwrite=True)
                    si.on_wait = [on_wait[-1]]
                    ins.sync_info = si
                new_list.append(ins)
            if changed:
                live = bb.instructions
                live[:] = new_list


def _re_ap(t, dims):
    """AP over tile/AP `t` with custom free dims (strides in elements)."""
    return bass.AP(tensor=t.tensor, offset=t.offset, ap=[t.ap[0]] + dims)


def _build():
    nc = bass.Bass()
    value = nc.dram_tensor("value", [BC, N, D], FP32, kind="ExternalInput")
    mask_t = nc.dram_tensor("mask_t", [128, CH, BC], FP32, kind="ExternalInput")
    e1_t = nc.dram_tensor("e1_t", [D, BC], FP32, kind="ExternalInput")
    w_lhsT = nc.dram_tensor("w_lhsT", [D, D], FP32, kind="ExternalInput")
    b_col = nc.dram_tensor("b_col", [D, 1], FP32, kind="ExternalInput")
    wfu_in = nc.dram_tensor("wfu", [D, 2], FP32, kind="ExternalInput")
    wf_row_in = nc.dram_tensor("wf_row", [1, D], FP32, kind="ExternalInput")
    attb_in = nc.dram_tensor("attb", [1, 1], FP32, kind="ExternalInput")
    ident_in = nc.dram_tensor("ident", [128, 128], FP32, kind="ExternalInput")
    y = nc.dram_tensor("y", [BC, D], FP32, kind="ExternalOutput")

    with tile.TileContext(nc) as tc, ExitStack() as ctx:
        P = lambda **kw: ctx.enter_context(tc.tile_pool(**kw))
        sb = P(name="sb", bufs=1)                        # persistent singles
        tp = P(name="tp", bufs=5)                        # vs mult temporaries
        wk = P(name="wk", bufs=3)                        # small temporaries
        vt = P(name="vt", bufs=2)                        # PE-route V^T staging
        ps_tr = P(name="ps_tr", bufs=2, space="PSUM")    # PE-route transposes
        ps_pass = P(name="ps_pass", bufs=3, space="PSUM")  # o-pass acc + o^T
        ps_sm = P(name="ps_sm", bufs=3, space="PSUM")    # small matmul outs

        # ---- small params over the sync queue ----
        w_sb = sb.tile([D, D], FP32, tag="w_sb")
        nc.sync.dma_start(out=w_sb, in_=w_lhsT[:, :])
        bcol_sb = sb.tile([D, 1], FP32, tag="bcol")
        nc.sync.dma_start(out=bcol_sb, in_=b_col[:, :])
        wfu_sb = sb.tile([D, 2], FP32, tag="wfu")
        wfrow_sb = sb.tile([1, D], FP32, tag="wfrow")
        nc.sync.dma_start(out=wfrow_sb, in_=wf_row_in[:, :])
        attb_sb = sb.tile([1, 1], FP32, tag="attb")
        nc.sync.dma_start(out=attb_sb, in_=attb_in[:, :])
        identf = sb.tile([128, 128], FP32, tag="identf")
        u0 = sb.tile([D, BC], FP32, tag="u0")
        nc.sync.dma_start(out=u0, in_=e1_t[:, :])

        # ---- V loads first (gpsimd SWDGE, fp32 -> bf16 cast), mask after
        # the first pair so row-0 compute starts early ----
        v_sb = [sb.tile([128, CH, D], BF16, tag=f"v{b}", name=f"v{b}")
                for b in range(BC)]
        mask_sb = sb.tile([128, CH, BC], BF16, tag="mask")

        nc.gpsimd.dma_start(out=identf, in_=ident_in[:, :])
        nc.gpsimd.dma_start(out=wfu_sb, in_=wfu_in[:, :])
        nc.gpsimd.dma_start(
            out=v_sb[0], in_=value[0].rearrange("(p j) d -> p j d", p=128))
        nc.gpsimd.dma_start(
            out=v_sb[1], in_=value[1].rearrange("(p j) d -> p j d", p=128))
        nc.gpsimd.dma_start(out=mask_sb, in_=mask_t[:, :, :])
        for b in range(2, BC):
            src = value[b].rearrange("(p j) d -> p j d", p=128)
            if b == 15:
                nc.gpsimd.dma_start(out=v_sb[b][:, 0:CH // 2, :],
                                    in_=src[:, 0:CH // 2, :])
                nc.gpsimd.dma_start(out=v_sb[b][:, CH // 2:, :],
                                    in_=src[:, CH // 2:, :])
            else:
                nc.gpsimd.dma_start(out=v_sb[b], in_=src)

        # ---- derived constants ----
        ones_row = sb.tile([1, 128], FP32, tag="onesr")
        nc.vector.memset(ones_row, 1.0)
        ones128 = sb.tile([128, 128], FP32, tag="ones128")
        nc.vector.memset(ones128, 1.0)
        identb = sb.tile([128, 128], BF16, tag="identb")
        nc.vector.tensor_copy(identb, identf)
        zeros_bf = sb.tile([128, 128], BF16, tag="zerosb")
        nc.vector.memset(zeros_bf, 0.0)
        c60_rep = sb.tile([128, 1], FP32, tag="c60")
        nc.vector.memset(c60_rep, CLAMP)
        wf_col = sb.tile([D, 1], BF16, tag="wfcol")
        nc.vector.tensor_copy(wf_col, wfu_sb[:, 0:1])
        wu_rep = sb.tile([D, 128], FP32, tag="wurep")
        w1 = wfu_sb[:, 1:2]
        nc.vector.tensor_copy(wu_rep, bass.AP(tensor=w1.tensor,
                                              offset=w1.offset,
                                              ap=[w1.ap[0], [0, 128]]))
        wf_bc = sb.tile([128, D], BF16, tag="wfbc")
        attb_rep = sb.tile([128, 1], FP32, tag="attbr")
        attb60_rep = sb.tile([128, 1], FP32, tag="attb60")

        def emit_params():
            wfp = ps_pass.tile([128, 128], FP32, tag="pass")
            nc.tensor.matmul(wfp, lhsT=ones_row, rhs=wfrow_sb, start=True,
                             stop=True)
            nc.vector.tensor_copy(wf_bc, wfp)
            abp = ps_sm.tile([128, 1], FP32, tag="sm")
            nc.tensor.matmul(abp, lhsT=ones_row, rhs=attb_sb, start=True,
                             stop=True)
            nc.vector.tensor_copy(attb_rep, abp)
            nc.vector.tensor_scalar_add(attb60_rep, attb_rep, CLAMP)

        # ---- persistent chain state ----
        vs_sb = sb.tile([128, BC, CH], BF16, tag="vs")   # b-major
        Em = sb.tile([128, CH, BC], BF16, tag="Em")      # c-major
        num_h = [sb.tile([128, CH, BC], BF16, tag=f"num{h}", name=f"num{h}")
                 for h in range(3)]
        ascratch = sb.tile([128, 128], BF16, tag="ascratch")
        u_t = [[None] * 4 for _ in range(4)]
        ub_t = [[None] * 3 for _ in range(4)]
        recip_t = [[None] * 3 for _ in range(4)]
        acc_t = [[None] * 3 for _ in range(4)]

        # ---- ub0 for all groups (deferred; emitted after vs_pe(b0,b1)) ----
        def emit_ub0():
            for gi, (g0, gn) in enumerate(GROUPS):
                u_t[gi][0] = u0[:, g0:g0 + gn]
                lp = ps_sm.tile([D, G], FP32, tag="sm")
                nc.tensor.matmul(lp, lhsT=w_sb, rhs=u_t[gi][0], start=True,
                                 stop=True)
                ub = sb.tile([D, G], FP32, tag=f"ub_g{gi}_h0")
                nc.scalar.activation(out=ub, in_=lp, func=AF.Relu,
                                     bias=bcol_sb, scale=1.0)
                ub_t[gi][0] = ub

        # ---- vs production pieces ----
        def vs_mult(b):
            tmpv = tp.tile([128, CH, D], BF16, tag="tmpv")
            nc.vector.tensor_tensor(
                out=tmpv, in0=v_sb[b],
                in1=_re_ap(wf_bc, [[0, CH], [1, D]]),
                op=ALU.mult)
            return tmpv

        def vs_red_dve(b, tmpv, c0=0, cn=CH):
            with nc.allow_low_precision(reason="vs bf16 out, fp32 accum"):
                nc.vector.tensor_reduce(
                    out=vs_sb[:, b, c0:c0 + cn],
                    in_=tmpv[:, c0:c0 + cn, :] if cn != CH else tmpv,
                    axis=mybir.AxisListType.X, op=ALU.add)

        def vs_red_act(b, tmpv):
            with nc.allow_low_precision(reason="vs bf16 accum"):
                for c in range(CH):
                    nc.scalar.activation(
                        out=ascratch, in_=tmpv[:, c, :], func=AF.Copy,
                        accum_out=vs_sb[:, b, c:c + 1])

        def vs_dve_halves(b):
            tmpv = tp.tile([128, CH, D], BF16, tag="tmpv")
            for half in range(2):
                c0 = half * (CH // 2)
                nc.vector.tensor_tensor(
                    out=tmpv[:, c0:c0 + CH // 2, :],
                    in0=v_sb[b][:, c0:c0 + CH // 2, :],
                    in1=_re_ap(wf_bc, [[0, CH // 2], [1, D]]),
                    op=ALU.mult)
                vs_red_dve(b, tmpv, c0, CH // 2)

        def vs_pe(b):
            # bf16 transposes (4 chunks per PSUM bank), copy to SBUF split
            # ACT/DVE, project onto wf. Software-pipelined over ps_tr bufs.
            acc_v = ps_sm.tile([128, CH], FP32, tag="sm")
            vt4s = [None, None]

            def emit_tr(cg):
                tr = ps_tr.tile([128, 512], BF16, tag="tr")
                for i in range(4):
                    c = cg * 4 + i
                    nc.tensor.transpose(out=tr[:, i * 128:(i + 1) * 128],
                                        in_=v_sb[b][:, c, :],
                                        identity=identb)
                vt4 = vt.tile([128, 512], BF16, tag="vt4")
                if cg % 2 == 0:
                    nc.scalar.activation(out=vt4, in_=tr, func=AF.Copy)
                else:
                    nc.vector.tensor_copy(vt4, tr)
                vt4s[cg % 2] = vt4

            def emit_proj(cg):
                vt4 = vt4s[cg % 2]
                for i in range(4):
                    c = cg * 4 + i
                    nc.tensor.matmul(acc_v[:, c:c + 1],
                                     lhsT=vt4[:, i * 128:(i + 1) * 128],
                                     rhs=wf_col, start=True, stop=True)

            emit_tr(0)
            emit_tr(1)
            emit_proj(0)
            emit_tr(2)
            emit_proj(1)
            emit_tr(3)
            emit_proj(2)
            emit_proj(3)
            with nc.allow_low_precision(reason="vs bf16 store"):
                nc.vector.tensor_copy(vs_sb[:, b, :], acc_v)

        def vs_pieces(b):
            """Return list of emission thunks for row b's vs, finest first."""
            if b in PE_VS:
                return [lambda: vs_pe(b)]
            if b in HALF_VS:
                return [lambda: vs_dve_halves(b)]
            st = {}

            def p1():
                st["t"] = vs_mult(b)

            if b in ACT_RED:
                return [p1, lambda: vs_red_act(b, st["t"])]
            return [p1, lambda: vs_red_dve(b, st["t"])]

        # ---- per-group stages ----
        def pre(gi, g0, gn):
            gsl = slice(g0, g0 + gn)
            vsl = vs_sb[:, gsl, :]
            vs_cb = _re_ap(vsl, [[1, CH], [CH, gn]])  # (c, b) iteration
            maskg = mask_sb[:, :, gsl]
            Etmp = wk.tile([128, CH, G], BF16, tag="Etmp")
            nc.scalar.activation(out=Etmp, in_=vs_cb, func=AF.Exp)
            nc.vector.tensor_tensor(out=Em[:, :, gsl], in0=Etmp, in1=maskg,
                                    op=ALU.mult)

        def chain_a(gi, g0, gn, h):
            gsl = slice(g0, g0 + gn)
            c_ps = ps_sm.tile([128, G], FP32, tag="sm")
            nc.tensor.matmul(c_ps, lhsT=wu_rep, rhs=u_t[gi][h], start=True,
                             stop=True)
            # exp(min(c,60)+attb) = exp((60+attb) - relu(60 - c))
            rc_sb = wk.tile([128, G], FP32, tag="rcs")
            nc.scalar.activation(out=rc_sb, in_=c_ps, func=AF.Relu,
                                 bias=c60_rep, scale=-1.0)
            tg = sb.tile([128, G], BF16, tag=f"tg{gi}_{h}")
            nc.scalar.activation(out=tg, in_=rc_sb, func=AF.Exp,
                                 bias=attb60_rep, scale=-1.0)
            tmpn = wk.tile([128, CH, G], BF16, tag="tmpn")
            nc.vector.tensor_tensor(
                out=tmpn, in0=Em[:, :, gsl],
                in1=_re_ap(tg, [[0, CH], [1, gn]]),
                op=ALU.mult)
            nc.vector.tensor_tensor(out=num_h[h][:, :, gsl], in0=tmpn,
                                    in1=mask_sb[:, :, gsl], op=ALU.max)

        def pass_denom(gi, g0, gn, h):
            acc = ps_pass.tile([128, 128], FP32, tag="pass")
            acc_t[gi][h] = acc
            # zero the whole bank via the PE so the later full-tile copy and
            # transpose never touch stale (potentially non-finite) PSUM data
            nc.tensor.matmul(acc, lhsT=zeros_bf, rhs=identb, start=True,
                             stop=True, skip_group_check=True)
            for c in range(CH):
                for j in range(gn):
                    b = g0 + j
                    nc.tensor.matmul(
                        acc[32 * j:32 * j + 1, :],
                        lhsT=num_h[h][:, c, b:b + 1],
                        rhs=v_sb[b][:, c, :],
                        start=(c == 0), stop=(c == CH - 1),
                        tile_position=(0, 32 * j),
                        skip_group_check=True)
            nsl = num_h[h][:, :, g0:g0 + gn]
            red = wk.tile([128, G], FP32, tag="red")
            nc.vector.tensor_reduce(
                out=red, in_=_re_ap(nsl, [[1, gn], [BC, CH]]),
                axis=mybir.AxisListType.X, op=ALU.add)
            dp = ps_sm.tile([128, G], FP32, tag="sm")
            nc.tensor.matmul(dp, lhsT=ones128, rhs=red, start=True, stop=True)
            recip = sb.tile([128, G], FP32, tag=f"recip{gi}_{h}")
            nc.vector.reciprocal(recip, dp)
            recip_t[gi][h] = recip

        def update(gi, g0, gn, h):
            acc = acc_t[gi][h]
            oa = wk.tile([128, 128], FP32, tag="oa")
            nc.scalar.activation(out=oa, in_=acc, func=AF.Copy)
            ot = ps_pass.tile([128, 128], FP32, tag="pass")
            nc.tensor.transpose(out=ot, in_=oa, identity=identf)
            onr = wk.tile([128, G], FP32, tag="onr")
            nc.vector.tensor_tensor(out=onr, in0=_re_ap(ot, [[32, G]]),
                                    in1=recip_t[gi][h], op=ALU.mult)
            un = sb.tile([D, G], FP32, tag=f"u_g{gi}_h{h + 1}")
            nc.vector.tensor_tensor(out=un, in0=onr, in1=ub_t[gi][h],
                                    op=ALU.add)
            u_t[gi][h + 1] = un
            if h < 2:
                lp = ps_sm.tile([D, G], FP32, tag="sm")
                nc.tensor.matmul(lp, lhsT=w_sb, rhs=un, start=True, stop=True)
                ub = sb.tile([D, G], FP32, tag=f"ub_g{gi}_h{h + 1}")
                nc.scalar.activation(out=ub, in_=lp, func=AF.Relu,
                                     bias=bcol_sb, scale=1.0)
                ub_t[gi][h + 1] = ub

        def finish(gi, g0, gn):
            yt = ps_sm.tile([G, 128], FP32, tag="sm")
            nc.tensor.transpose(out=yt, in_=u_t[gi][3], identity=identf)
            yg = wk.tile([G, 128], FP32, tag="yg")
            nc.vector.tensor_copy(yg, yt)
            nc.sync.dma_start(out=y[g0:g0 + gn, :], in_=yg)

        # ---- software pipeline ----
        def emit_group_chain(gi, splice):
            g0, gn = GROUPS[gi]
            si = 0

            def do_splice(k):
                nonlocal si
                while si < min(k, len(splice)):
                    splice[si]()
                    si += 1

            pre(gi, g0, gn)
            for h in range(3):
                chain_a(gi, g0, gn, h)
                do_splice((h + 1) * len(splice) // 4)
                pass_denom(gi, g0, gn, h)
                do_splice((h + 1) * len(splice) // 3)
                update(gi, g0, gn, h)
            do_splice(len(splice))
            finish(gi, g0, gn)

        vs_pe(0)
        vs_pe(1)
        emit_params()
        emit_ub0()
        vs_pe(2)
        vs_pe(3)
        sp1 = [p for b in range(4, 8) for p in vs_pieces(b)]
        sp2 = [p for b in range(8, 12) for p in vs_pieces(b)]
        sp3 = [p for b in range(12, 16) for p in vs_pieces(b)]
        emit_group_chain(0, sp1)
        emit_group_chain(1, sp2)
        emit_group_chain(2, sp3)
        emit_group_chain(3, [])

    _split_multiwaits(nc)
    return nc


_nc_cache = None


def _get_nc():
    global _nc_cache
    if _nc_cache is None:
        _nc_cache = _build()
    return _nc_cache


def make_in_maps(inputs):
    e1 = np.asarray(inputs["e1_embeded"], dtype=np.float32)
    value = np.asarray(inputs["nei_embeded_value"], dtype=np.float32)
    mask = np.asarray(inputs["nei_mask"], dtype=np.float32)
    linfc_w = np.asarray(inputs["linfc_w"], dtype=np.float32)
    linfc_b = np.asarray(inputs["linfc_b"], dtype=np.float32)
    attfc_w = np.asarray(inputs["attfc_w"], dtype=np.float32)
    attfc_b = np.asarray(inputs["attfc_b"], dtype=np.float32)

    w_lhsT = np.ascontiguousarray(linfc_w.T)
    b_colv = np.ascontiguousarray(linfc_b.reshape(D, 1))
    wfu = np.ascontiguousarray(
        np.stack([attfc_w[0, :D], attfc_w[0, D:]], axis=1))
    wf_row = np.ascontiguousarray(attfc_w[0:1, :D])
    attb = np.asarray(attfc_b, dtype=np.float32).reshape(1, 1)
    ident = np.eye(128, dtype=np.float32)

    in_maps = []
    for core in range(N_CORES):
        b0 = core * BC
        in_maps.append({
            "value": np.ascontiguousarray(value[b0:b0 + BC]),
            "mask_t": np.ascontiguousarray(np.transpose(
                mask[b0:b0 + BC].reshape(BC, 128, CH), (1, 2, 0))),
            "e1_t": np.ascontiguousarray(e1[b0:b0 + BC].T),
            "w_lhsT": w_lhsT,
            "b_col": b_colv,
            "wfu": wfu,
            "wf_row": wf_row,
            "attb": attb,
            "ident": ident,
        })
    return in_maps


def kernel(**inputs):
    in_maps = make_in_maps(inputs)
    nc = _get_nc()
    res = run_bass_kernel_spmd(nc, in_maps, list(range(N_CORES)))
    out = np.concatenate([res.results[i]["y"] for i in range(N_CORES)], axis=0)
    return out.astype(np.float32)


# revision 22
# speedup vs baseline: 1.3040x; 1.3040x over previous
"""DMN encoder (3-hop masked-attention message passing) on 8 trn2 cores.

Data-parallel over batch (16 rows/core). Per core, per batch row b:
  vs_n = V[b] @ wf  (hop-invariant attention projection)
  3 hops: num_h = mask*max(exp(vs)*exp(c_h), 1);  o_h = num_h^T V / denom_h
          u_{h+1} = relu(W u_h + bias) + o_h

Structure notes:
  - 3 o-passes (one per hop) instead of 2: removes the vu = V@wu
    per-neighbor projection entirely.
  - The reference's 1e-5*exp(max_att) denominator guard is dropped: the
    denominator always contains the max kept exponential, so the guard is
    bounded by ~1e-5 relative — far below the accuracy target.
  - vs is produced on whichever engine has slack when that row's V tile
    arrives: PE transpose+project (rows 0-3 early, 12-13 late), gpsimd
    multiply (rows 4,6,8,10) with the reduce on ACT/DVE, DVE for the rest.
  - o-pass matmuls are column-tiled 4-wide (tile_position=(0,32j)): four
    batch rows stream through the PE concurrently.
  - Hop scalars stay replicated across partitions (wu_rep / ones128
    matmuls) so the serial chain has no broadcast steps; tensor_scalar
    with AP scalars (TensorScalarPtr, ~9us on 128 partitions) is avoided.
"""
import sys

sys.path.insert(0, "/opt/trn_rl_repo")

import numpy as np
import concourse.bass as bass
import concourse.tile as tile
from concourse import mybir
from concourse.bass_utils import run_bass_kernel_spmd
from contextlib import ExitStack

N_CORES = 8
B, N, D = 128, 2048, 128
BC = B // N_CORES          # batch rows per core
CH = N // 128              # neighbor chunks of 128
AF = mybir.ActivationFunctionType
ALU = mybir.AluOpType
FP32 = mybir.dt.float32
BF16 = mybir.dt.bfloat16
CLAMP = 60.0               # overflow guard on exp() arguments

G = 4                       # rows per chain group
GROUPS = [(0, G), (4, G), (8, G), (12, G)]
PE_VS = (0, 1, 2, 3, 12, 13)   # rows: vs via PE transpose route
ACT_RED = (4, 6, 8, 10)        # rows: vs reduce via ACT accumulate
HALF_VS = (15,)                # rows: DVE vs in half-tile pieces (tail)

_mwctr = [0]


def _split_multiwaits(nc):
    """This walrus build rejects >1 sync-wait per instruction; hoist extras
    onto standalone EventSemaphore instructions on the same engine."""
    for fn in nc.m.functions:
        for bb in fn.blocks:
            new_list = []
            changed = False
            for ins in bb.instructions:
                si = getattr(ins, "sync_info", None)
                on_wait = list(si.on_wait) if si is not None else []
                if len(on_wait) > 1:
                    changed = True
                    for w in on_wait[:-1]:
                        _mwctr[0] += 1
                        ev = mybir.InstEventSemaphore(
                            name=f"I-mwfix-{_mwctr[0]}", ins=[], outs=[])
                        ev.engine = ins.engine
                        ev.debug = ins.debug
                        ev.sync_info = mybir.SyncInfo(on_wait=[w], on_update=[])
                        new_list.append(ev)
                        nc.register_instruction(ev, overwrite=True)
                    si.on_wait = [on_wait[-1]]
                    ins.sync_info = si
                new_list.append(ins)
            if changed:
                live = bb.instructions
                live[:] = new_list


def _re_ap(t, dims):
    """AP over tile/AP `t` with custom free dims (strides in elements)."""
    return bass.AP(tensor=t.tensor, offset=t.offset, ap=[t.ap[0]] + dims)


def _build():
    nc = bass.Bass()
    value = nc.dram_tensor("value", [BC, N, D], FP32, kind="ExternalInput")
    mask_t = nc.dram_tensor("mask_t", [128, CH, BC], FP32, kind="ExternalInput")
    e1_t = nc.dram_tensor("e1_t", [D, BC], FP32, kind="ExternalInput")
    w_lhsT = nc.dram_tensor("w_lhsT", [D, D], FP32, kind="ExternalInput")
    b_col = nc.dram_tensor("b_col", [D, 1], FP32, kind="ExternalInput")
    wfu_in = nc.dram_tensor("wfu", [D, 2], FP32, kind="ExternalInput")
    wf_row_in = nc.dram_tensor("wf_row", [1, D], FP32, kind="ExternalInput")
    attb_in = nc.dram_tensor("attb", [1, 1], FP32, kind="ExternalInput")
    ident_in = nc.dram_tensor("ident", [128, 128], FP32, kind="ExternalInput")
    y = nc.dram_tensor("y", [BC, D], FP32, kind="ExternalOutput")

    with tile.TileContext(nc) as tc, ExitStack() as ctx:
        P = lambda **kw: ctx.enter_context(tc.tile_pool(**kw))
        sb = P(name="sb", bufs=1)                        # persistent singles
        tp = P(name="tp", bufs=5)                        # vs mult temporaries
        wk = P(name="wk", bufs=3)                        # small temporaries
        vt = P(name="vt", bufs=2)                        # PE-route V^T staging
        ps_tr = P(name="ps_tr", bufs=2, space="PSUM")    # PE-route transposes
        ps_pass = P(name="ps_pass", bufs=3, space="PSUM")  # o-pass acc + o^T
        ps_sm = P(name="ps_sm", bufs=3, space="PSUM")    # small matmul outs

        # ---- small params over the sync queue ----
        w_sb = sb.tile([D, D], FP32, tag="w_sb")
        nc.sync.dma_start(out=w_sb, in_=w_lhsT[:, :])
        bcol_sb = sb.tile([D, 1], FP32, tag="bcol")
        nc.sync.dma_start(out=bcol_sb, in_=b_col[:, :])
        wfu_sb = sb.tile([D, 2], FP32, tag="wfu")
        wfrow_sb = sb.tile([1, D], FP32, tag="wfrow")
        nc.sync.dma_start(out=wfrow_sb, in_=wf_row_in[:, :])
        attb_sb = sb.tile([1, 1], FP32, tag="attb")
        nc.sync.dma_start(out=attb_sb, in_=attb_in[:, :])
        identf = sb.tile([128, 128], FP32, tag="identf")
        u0 = sb.tile([D, BC], FP32, tag="u0")
        nc.sync.dma_start(out=u0, in_=e1_t[:, :])

        # ---- V loads first (gpsimd SWDGE, fp32 -> bf16 cast), mask after
        # the first pair so row-0 compute starts early ----
        v_sb = [sb.tile([128, CH, D], BF16, tag=f"v{b}", name=f"v{b}")
                for b in range(BC)]
        mask_sb = sb.tile([128, CH, BC], BF16, tag="mask")

        nc.gpsimd.dma_start(out=identf, in_=ident_in[:, :])
        nc.gpsimd.dma_start(out=wfu_sb, in_=wfu_in[:, :])
        nc.gpsimd.dma_start(
            out=v_sb[0], in_=value[0].rearrange("(p j) d -> p j d", p=128))
        nc.gpsimd.dma_start(
            out=v_sb[1], in_=value[1].rearrange("(p j) d -> p j d", p=128))
        nc.gpsimd.dma_start(out=mask_sb, in_=mask_t[:, :, :])
        for b in range(2, BC):
            src = value[b].rearrange("(p j) d -> p j d", p=128)
            if b == 15:
                nc.gpsimd.dma_start(out=v_sb[b][:, 0:CH // 2, :],
                                    in_=src[:, 0:CH // 2, :])
                nc.gpsimd.dma_start(out=v_sb[b][:, CH // 2:, :],
                                    in_=src[:, CH // 2:, :])
            else:
                nc.gpsimd.dma_start(out=v_sb[b], in_=src)

        # ---- derived constants ----
        ones_row = sb.tile([1, 128], FP32, tag="onesr")
        nc.vector.memset(ones_row, 1.0)
        ones128 = sb.tile([128, 128], FP32, tag="ones128")
        nc.vector.memset(ones128, 1.0)
        identb = sb.tile([128, 128], BF16, tag="identb")
        nc.vector.tensor_copy(identb, identf)
        zeros_bf = sb.tile([128, 128], BF16, tag="zerosb")
        nc.vector.memset(zeros_bf, 0.0)
        c60_rep = sb.tile([128, 1], FP32, tag="c60")
        nc.vector.memset(c60_rep, CLAMP)
        wf_col = sb.tile([D, 1], BF16, tag="wfcol")
        nc.vector.tensor_copy(wf_col, wfu_sb[:, 0:1])
        wu_rep = sb.tile([D, 128], FP32, tag="wurep")
        w1 = wfu_sb[:, 1:2]
        nc.vector.tensor_copy(wu_rep, bass.AP(tensor=w1.tensor,
                                              offset=w1.offset,
                                              ap=[w1.ap[0], [0, 128]]))
        wf_bc = sb.tile([128, D], BF16, tag="wfbc")
        attb_rep = sb.tile([128, 1], FP32, tag="attbr")
        attb60_rep = sb.tile([128, 1], FP32, tag="attb60")

        def emit_params():
            wfp = ps_pass.tile([128, 128], FP32, tag="pass")
            nc.tensor.matmul(wfp, lhsT=ones_row, rhs=wfrow_sb, start=True,
                             stop=True)
            nc.vector.tensor_copy(wf_bc, wfp)
            abp = ps_sm.tile([128, 1], FP32, tag="sm")
            nc.tensor.matmul(abp, lhsT=ones_row, rhs=attb_sb, start=True,
                             stop=True)
            nc.vector.tensor_copy(attb_rep, abp)
            nc.vector.tensor_scalar_add(attb60_rep, attb_rep, CLAMP)

        # ---- persistent chain state ----
        vs_sb = sb.tile([128, BC, CH], BF16, tag="vs")   # b-major
        Em = sb.tile([128, CH, BC], BF16, tag="Em")      # c-major
        num_h = [sb.tile([128, CH, BC], BF16, tag=f"num{h}", name=f"num{h}")
                 for h in range(3)]
        ascratch = sb.tile([128, 128], BF16, tag="ascratch")
        u_t = [[None] * 4 for _ in range(4)]
        ub_t = [[None] * 3 for _ in range(4)]
        recip_t = [[None] * 3 for _ in range(4)]
        acc_t = [[None] * 3 for _ in range(4)]

        # ---- ub0 for all groups (deferred; emitted after vs_pe(b0,b1)) ----
        def emit_ub0():
            for gi, (g0, gn) in enumerate(GROUPS):
                u_t[gi][0] = u0[:, g0:g0 + gn]
                lp = ps_sm.tile([D, G], FP32, tag="sm")
                nc.tensor.matmul(lp, lhsT=w_sb, rhs=u_t[gi][0], start=True,
                                 stop=True)
                ub = sb.tile([D, G], FP32, tag=f"ub_g{gi}_h0")
                nc.scalar.activation(out=ub, in_=lp, func=AF.Relu,
                                     bias=bcol_sb, scale=1.0)
                ub_t[gi][0] = ub

        # ---- vs production pieces ----
        def vs_mult(b):
            tmpv = tp.tile([128, CH, D], BF16, tag="tmpv")
            nc.vector.tensor_tensor(
                out=tmpv, in0=v_sb[b],
                in1=_re_ap(wf_bc, [[0, CH], [1, D]]),
                op=ALU.mult)
            return tmpv

        def vs_red_dve(b, tmpv, c0=0, cn=CH):
            with nc.allow_low_precision(reason="vs bf16 out, fp32 accum"):
                nc.vector.tensor_reduce(
                    out=vs_sb[:, b, c0:c0 + cn],
                    in_=tmpv[:, c0:c0 + cn, :] if cn != CH else tmpv,
                    axis=mybir.AxisListType.X, op=ALU.add)

        def vs_red_act(b, tmpv):
            with nc.allow_low_precision(reason="vs bf16 accum"):
                for c in range(CH):
                    nc.scalar.activation(
                        out=ascratch, in_=tmpv[:, c, :], func=AF.Copy,
                        accum_out=vs_sb[:, b, c:c + 1])

        def vs_dve_halves(b):
            tmpv = tp.tile([128, CH, D], BF16, tag="tmpv")
            for half in range(2):
                c0 = half * (CH // 2)
                nc.vector.tensor_tensor(
                    out=tmpv[:, c0:c0 + CH // 2, :],
                    in0=v_sb[b][:, c0:c0 + CH // 2, :],
                    in1=_re_ap(wf_bc, [[0, CH // 2], [1, D]]),
                    op=ALU.mult)
                vs_red_dve(b, tmpv, c0, CH // 2)

        def vs_pe(b):
            # bf16 transposes (4 chunks per PSUM bank), copy to SBUF split
            # ACT/DVE, project onto wf. Software-pipelined over ps_tr bufs.
            acc_v = ps_sm.tile([128, CH], FP32, tag="sm")
            vt4s = [None, None]

            def emit_tr(cg):
                tr = ps_tr.tile([128, 512], BF16, tag="tr")
                for i in range(4):
                    c = cg * 4 + i
                    nc.tensor.transpose(out=tr[:, i * 128:(i + 1) * 128],
                                        in_=v_sb[b][:, c, :],
                                        identity=identb)
                vt4 = vt.tile([128, 512], BF16, tag="vt4")
                if cg % 2 == 0:
                    nc.scalar.activation(out=vt4, in_=tr, func=AF.Copy)
                else:
                    nc.vector.tensor_copy(vt4, tr)
                vt4s[cg % 2] = vt4

            def emit_proj(cg):
                vt4 = vt4s[cg % 2]
                for i in range(4):
                    c = cg * 4 + i
                    nc.tensor.matmul(acc_v[:, c:c + 1],
                                     lhsT=vt4[:, i * 128:(i + 1) * 128],
                                     rhs=wf_col, start=True, stop=True)

            emit_tr(0)
            emit_tr(1)
            emit_proj(0)
            emit_tr(2)
            emit_proj(1)
            emit_tr(3)
            emit_proj(2)
            emit_proj(3)
            with nc.allow_low_precision(reason="vs bf16 store"):
                nc.vector.tensor_copy(vs_sb[:, b, :], acc_v)

        def vs_pieces(b):
            """Return list of emission thunks for row b's vs, finest first."""
            if b in PE_VS:
                return [lambda: vs_pe(b)]
            if b in HALF_VS:
                return [lambda: vs_dve_halves(b)]
            st = {}

            def p1():
                st["t"] = vs_mult(b)

            if b in ACT_RED:
                return [p1, lambda: vs_red_act(b, st["t"])]
            return [p1, lambda: vs_red_dve(b, st["t"])]

        # ---- per-group stages ----
        def pre(gi, g0, gn):
            gsl = slice(g0, g0 + gn)
            vsl = vs_sb[:, gsl, :]
            vs_cb = _re_ap(vsl, [[1, CH], [CH, gn]])  # (c, b) iteration
            maskg = mask_sb[:, :, gsl]
            Etmp = wk.tile([128, CH, G], BF16, tag="Etmp")
            nc.scalar.activation(out=Etmp, in_=vs_cb, func=AF.Exp)
            nc.vector.tensor_tensor(out=Em[:, :, gsl], in0=Etmp, in1=maskg,
                                    op=ALU.mult)

        def chain_a(gi, g0, gn, h):
            gsl = slice(g0, g0 + gn)
            c_ps = ps_sm.tile([128, G], FP32, tag="sm")
            nc.tensor.matmul(c_ps, lhsT=wu_rep, rhs=u_t[gi][h], start=True,
                             stop=True)
            # exp(min(c,60)+attb) = exp((60+attb) - relu(60 - c))
            rc_sb = wk.tile([128, G], FP32, tag="rcs")
            nc.scalar.activation(out=rc_sb, in_=c_ps, func=AF.Relu,
                                 bias=c60_rep, scale=-1.0)
            tg = sb.tile([128, G], BF16, tag=f"tg{gi}_{h}")
            nc.scalar.activation(out=tg, in_=rc_sb, func=AF.Exp,
                                 bias=attb60_rep, scale=-1.0)
            tmpn = wk.tile([128, CH, G], BF16, tag="tmpn")
            nc.vector.tensor_tensor(
                out=tmpn, in0=Em[:, :, gsl],
                in1=_re_ap(tg, [[0, CH], [1, gn]]),
                op=ALU.mult)
            nc.vector.tensor_tensor(out=num_h[h][:, :, gsl], in0=tmpn,
                                    in1=mask_sb[:, :, gsl], op=ALU.max)

        def pass_denom(gi, g0, gn, h):
            acc = ps_pass.tile([128, 128], FP32, tag="pass")
            acc_t[gi][h] = acc
            # zero the whole bank via the PE so the later full-tile copy and
            # transpose never touch stale (potentially non-finite) PSUM data
            nc.tensor.matmul(acc, lhsT=zeros_bf, rhs=identb, start=True,
                             stop=True, skip_group_check=True)
            for c in range(CH):
                for j in range(gn):
                    b = g0 + j
                    nc.tensor.matmul(
                        acc[32 * j:32 * j + 1, :],
                        lhsT=num_h[h][:, c, b:b + 1],
                        rhs=v_sb[b][:, c, :],
                        start=(c == 0), stop=(c == CH - 1),
                        tile_position=(0, 32 * j),
                        skip_group_check=True)
            nsl = num_h[h][:, :, g0:g0 + gn]
            red = wk.tile([128, G], FP32, tag="red")
            nc.vector.tensor_reduce(
                out=red, in_=_re_ap(nsl, [[1, gn], [BC, CH]]),
                axis=mybir.AxisListType.X, op=ALU.add)
            dp = ps_sm.tile([128, G], FP32, tag="sm")
            nc.tensor.matmul(dp, lhsT=ones128, rhs=red, start=True, stop=True)
            recip = sb.tile([128, G], FP32, tag=f"recip{gi}_{h}")
            nc.vector.reciprocal(recip, dp)
            recip_t[gi][h] = recip

        def update(gi, g0, gn, h):
            acc = acc_t[gi][h]
            oa = wk.tile([128, 128], FP32, tag="oa")
            nc.scalar.activation(out=oa, in_=acc, func=AF.Copy)
            ot = ps_pass.tile([128, 128], FP32, tag="pass")
            nc.tensor.transpose(out=ot, in_=oa, identity=identf)
            onr = wk.tile([128, G], FP32, tag="onr")
            nc.vector.tensor_tensor(out=onr, in0=_re_ap(ot, [[32, G]]),
                                    in1=recip_t[gi][h], op=ALU.mult)
            un = sb.tile([D, G], FP32, tag=f"u_g{gi}_h{h + 1}")
            nc.vector.tensor_tensor(out=un, in0=onr, in1=ub_t[gi][h],
                                    op=ALU.add)
            u_t[gi][h + 1] = un
            if h < 2:
                lp = ps_sm.tile([D, G], FP32, tag="sm")
                nc.tensor.matmul(lp, lhsT=w_sb, rhs=un, start=True, stop=True)
                ub = sb.tile([D, G], FP32, tag=f"ub_g{gi}_h{h + 1}")
                nc.scalar.activation(out=ub, in_=lp, func=AF.Relu,
                                     bias=bcol_sb, scale=1.0)
                ub_t[gi][h + 1] = ub

        def finish(gi, g0, gn):
            yt = ps_sm.tile([G, 128], FP32, tag="sm")
            nc.tensor.transpose(out=yt, in_=u_t[gi][3], identity=identf)
            yg = wk.tile([G, 128], FP32, tag="yg")
            nc.vector.tensor_copy(yg, yt)
            nc.sync.dma_start(out=y[g0:g0 + gn, :], in_=yg)

        # ---- software pipeline: skewed wavefront emission so adjacent
        # groups' chains overlap through the in-order engine queues ----
        vs_pe(0)
        vs_pe(1)
        emit_params()
        emit_ub0()
        vs_pe(2)
        vs_pe(3)

        SKEW = 3
        events = []

        def add(key, fn):
            events.append((key, len(events), fn))

        for gi in range(4):
            g0, gn = GROUPS[gi]
            add(SKEW * gi + 0.0,
                lambda gi=gi, g0=g0, gn=gn: pre(gi, g0, gn))
            for h in range(3):
                add(SKEW * gi + 1 + 3 * h,
                    lambda gi=gi, g0=g0, gn=gn, h=h: chain_a(gi, g0, gn, h))
                add(SKEW * gi + 2 + 3 * h,
                    lambda gi=gi, g0=g0, gn=gn, h=h: pass_denom(gi, g0, gn, h))
                add(SKEW * gi + 3 + 3 * h,
                    lambda gi=gi, g0=g0, gn=gn, h=h: update(gi, g0, gn, h))
            add(SKEW * gi + 10.0,
                lambda gi=gi, g0=g0, gn=gn: finish(gi, g0, gn))
        for gi in (1, 2, 3):
            g0 = GROUPS[gi][0]
            pieces = [p for b in range(g0, g0 + 4) for p in vs_pieces(b)]
            for i, p in enumerate(pieces):
                add(SKEW * gi - 2.75 + 0.3 * i, p)
        for key, idx, fn in sorted(events):
            fn()

    _split_multiwaits(nc)
    return nc


_nc_cache = None


def _get_nc():
    global _nc_cache
    if _nc_cache is None:
        _nc_cache = _build()
    return _nc_cache


def make_in_maps(inputs):
    e1 = np.asarray(inputs["e1_embeded"], dtype=np.float32)
    value = np.asarray(inputs["nei_embeded_value"], dtype=np.float32)
    mask = np.asarray(inputs["nei_mask"], dtype=np.float32)
    linfc_w = np.asarray(inputs["linfc_w"], dtype=np.float32)
    linfc_b = np.asarray(inputs["linfc_b"], dtype=np.float32)
    attfc_w = np.asarray(inputs["attfc_w"], dtype=np.float32)
    attfc_b = np.asarray(inputs["attfc_b"], dtype=np.float32)

    w_lhsT = np.ascontiguousarray(linfc_w.T)
    b_colv = np.ascontiguousarray(linfc_b.reshape(D, 1))
    wfu = np.ascontiguousarray(
        np.stack([attfc_w[0, :D], attfc_w[0, D:]], axis=1))
    wf_row = np.ascontiguousarray(attfc_w[0:1, :D])
    attb = np.asarray(attfc_b, dtype=np.float32).reshape(1, 1)
    ident = np.eye(128, dtype=np.float32)

    in_maps = []
    for core in range(N_CORES):
        b0 = core * BC
        in_maps.append({
            "value": np.ascontiguousarray(value[b0:b0 + BC]),
            "mask_t": np.ascontiguousarray(np.transpose(
                mask[b0:b0 + BC].reshape(BC, 128, CH), (1, 2, 0))),
            "e1_t": np.ascontiguousarray(e1[b0:b0 + BC].T),
            "w_lhsT": w_lhsT,
            "b_col": b_colv,
            "wfu": wfu,
            "wf_row": wf_row,
            "attb": attb,
            "ident": ident,
        })
    return in_maps


def kernel(**inputs):
    in_maps = make_in_maps(inputs)
    nc = _get_nc()
    res = run_bass_kernel_spmd(nc, in_maps, list(range(N_CORES)))
    out = np.concatenate([res.results[i]["y"] for i in range(N_CORES)], axis=0)
    return out.astype(np.float32)
